# revision 8
# baseline (speedup 1.0000x reference)
"""Trainium2 Bass kernel for 2-layer GCN (nn_BasicGNN).

Strategy (8 NeuronCores, SPMD):
  - Reorder layer 2 as (A_norm @ z1) @ W2 so both aggregations move 16 feats.
  - out[v] = dinv[v] * (sum_{u->v} dinv[u]*h[u] + dinv[v]*h[v])  (self-loop)
    => per-node pre-scale by dinv, aggregate raw sums, post-scale by dinv.
  - Shard destinations across 8 cores (12500 each). Host sorts each core's
    dests by in-degree, pads edge lists per 128-dest group to the group max
    K_g, and maps edge sources to "table rows" (per-core slabs concatenated
    by AllGather).
  - Device per layer: gather [128, K_g*16] rows per group with one
    indirect DMA, strided reduce_sum on DVE, add self term, scale, next.
  - Weights (128x16, 16x40) replicated; AllGather shares the 16-feature
    tables between layers.
"""

import sys
import numpy as np

if "/opt/trn_rl_repo" not in sys.path:
    sys.path.insert(0, "/opt/trn_rl_repo")

N_CORES = 8
P = 128


def _preprocess(x, edge_index, W1, b1, W2, b2):
    x = np.asarray(x, dtype=np.float32)
    W1 = np.asarray(W1, dtype=np.float32)
    b1 = np.asarray(b1, dtype=np.float32)
    W2 = np.asarray(W2, dtype=np.float32)
    b2 = np.asarray(b2, dtype=np.float32)
    N, F_IN = x.shape
    F_HID = W1.shape[1]
    F_OUT = W2.shape[1]
    M = N_CORES
    assert N % M == 0
    Ns = N // M
    NsP = ((Ns + P - 1) // P) * P      # padded dest count per core
    G = NsP // P                        # dest groups per core
    S = NsP + P                         # slab rows (last P rows = zeros)
    PAD_ROW = NsP                       # table row that is guaranteed zero (core 0 zero block)

    row = np.asarray(edge_index[0]).astype(np.int64)
    col = np.asarray(edge_index[1]).astype(np.int64)
    deg = np.bincount(col, minlength=N).astype(np.int64) + 1
    dinv = (deg.astype(np.float64) ** -0.5).astype(np.float32)

    # Per-core degree-sorted dest permutation; node -> global table row
    pos_global = np.empty(N, dtype=np.int64)
    pos_of_list = []
    sorted_indeg = []
    for m in range(M):
        indeg = deg[m * Ns:(m + 1) * Ns] - 1
        key = np.concatenate([indeg, np.full(NsP - Ns, -1, dtype=np.int64)])
        order = np.argsort(key, kind="stable")
        pos_of = np.empty(NsP, dtype=np.int64)
        pos_of[order] = np.arange(NsP)
        pos_of_list.append(pos_of)
        sorted_indeg.append(np.maximum(key[order], 0))
        pos_global[m * Ns:(m + 1) * Ns] = m * S + pos_of[:Ns]

    # Shared per-group K (max over cores, >=1)
    Ks = np.zeros(G, dtype=np.int64)
    for m in range(M):
        si = sorted_indeg[m].reshape(G, P)
        Ks = np.maximum(Ks, si.max(axis=1))
    Ks = np.maximum(Ks, 1)
    offs = np.zeros(G + 1, dtype=np.int64)
    offs[1:] = np.cumsum(Ks)
    SUMK = int(offs[-1])

    in_maps = []
    for m in range(M):
        pos_of = pos_of_list[m]
        mask = (col >= m * Ns) & (col < (m + 1) * Ns)
        er = row[mask]
        dpos = pos_of[col[mask] - m * Ns]
        src_row = pos_global[er]
        o = np.argsort(dpos, kind="stable")
        dpos = dpos[o]
        src_row = src_row[o]
        cnt = np.bincount(dpos, minlength=NsP)
        starts = np.concatenate([[0], np.cumsum(cnt)])[:-1]
        rank = np.arange(len(dpos)) - starts[dpos]
        idx_all = np.full((P, SUMK), PAD_ROW, dtype=np.int32)
        g_of = dpos // P
        p_of = dpos % P
        idx_all[p_of, offs[g_of] + rank] = src_row.astype(np.int32)

        xp = np.zeros((NsP, F_IN), np.float32)
        xp[pos_of[:Ns]] = x[m * Ns:(m + 1) * Ns]
        xT = np.ascontiguousarray(xp.T)

        d_sorted = np.ones(NsP, np.float32)
        d_sorted[pos_of[:Ns]] = dinv[m * Ns:(m + 1) * Ns]
        dinv_col = np.ascontiguousarray(d_sorted.reshape(G, P).T)
        dinv2_col = np.ascontiguousarray(dinv_col * dinv_col)
        db1 = (d_sorted.reshape(G, P)[:, :, None] * b1[None, None, :])
        db1 = np.ascontiguousarray(db1.transpose(1, 0, 2).reshape(P, G * F_HID))

        in_maps.append({
            "xT": xT,
            "idx": idx_all,
            "dinv": dinv_col,
            "dinv2": dinv2_col,
            "db1": db1.astype(np.float32),
            "W1": W1,
            "W2": W2,
            "b2r": np.ascontiguousarray(np.tile(b2[None, :], (P, 1))),
        })

    meta = dict(N=N, Ns=Ns, NsP=NsP, G=G, S=S, Ks=Ks.tolist(), offs=offs.tolist(),
                SUMK=SUMK, F_IN=F_IN, F_HID=F_HID, F_OUT=F_OUT,
                pos_of_list=pos_of_list)
    return meta, in_maps


def _build_program(meta, dbg=False):
    import concourse.bacc as bacc
    import concourse.tile as tile
    import concourse.bass as bass
    import concourse.mybir as mybir
    from concourse.masks import make_identity

    f32 = mybir.dt.float32
    i32 = mybir.dt.int32
    G, S, NsP, SUMK = meta["G"], meta["S"], meta["NsP"], meta["SUMK"]
    Ks, offs = meta["Ks"], meta["offs"]
    F_IN, F_HID, F_OUT = meta["F_IN"], meta["F_HID"], meta["F_OUT"]
    M = N_CORES

    nc = bacc.Bacc("TRN2", target_bir_lowering=False, debug=False,
                   enable_asserts=False, num_devices=M)

    xT_d = nc.dram_tensor("xT", [P, NsP], f32, kind="ExternalInput")
    idx_d = nc.dram_tensor("idx", [P, SUMK], i32, kind="ExternalInput")
    dinv_d = nc.dram_tensor("dinv", [P, G], f32, kind="ExternalInput")
    dinv2_d = nc.dram_tensor("dinv2", [P, G], f32, kind="ExternalInput")
    db1_d = nc.dram_tensor("db1", [P, G * F_HID], f32, kind="ExternalInput")
    W1_d = nc.dram_tensor("W1", [F_IN, F_HID], f32, kind="ExternalInput")
    W2_d = nc.dram_tensor("W2", [F_HID, F_OUT], f32, kind="ExternalInput")
    b2r_d = nc.dram_tensor("b2r", [P, F_OUT], f32, kind="ExternalInput")
    out_d = nc.dram_tensor("out", [NsP, F_OUT], f32, kind="ExternalOutput")

    if dbg:
        dslab_d = nc.dram_tensor("dslab", [NsP, F_HID], f32, kind="ExternalOutput")
        dtab_d = nc.dram_tensor("dtab", [2048, F_HID], f32, kind="ExternalOutput")
        dagg_d = nc.dram_tensor("dagg", [NsP, F_HID], f32, kind="ExternalOutput")
    slab1 = nc.dram_tensor("slab1", [S, F_HID], f32, kind="Internal")
    slab2 = nc.dram_tensor("slab2", [S, F_HID], f32, kind="Internal")
    tab1 = nc.dram_tensor("tab1", [M * S, F_HID], f32, kind="Internal",
                          addr_space="Shared")
    tab2 = nc.dram_tensor("tab2", [M * S, F_HID], f32, kind="Internal",
                          addr_space="Shared")
    RG = [list(range(M))]

    with tile.TileContext(nc) as tc:
        with tc.tile_pool(name="big", bufs=1) as bigp, \
             tc.tile_pool(name="wts", bufs=1) as wp, \
             tc.tile_pool(name="work", bufs=6) as sb, \
             tc.tile_pool(name="gath", bufs=3) as gp, \
             tc.tile_pool(name="ps", bufs=2, space="PSUM") as pp:

            xT_s = bigp.tile([P, NsP], f32)
            nc.sync.dma_start(xT_s[:], xT_d[:])
            idx_s = bigp.tile([P, SUMK], i32)
            nc.sync.dma_start(idx_s[:], idx_d[:])
            dinv_s = wp.tile([P, G], f32)
            nc.sync.dma_start(dinv_s[:], dinv_d[:])
            dinv2_s = wp.tile([P, G], f32)
            nc.sync.dma_start(dinv2_s[:], dinv2_d[:])
            db1_s = wp.tile([P, G * F_HID], f32)
            nc.sync.dma_start(db1_s[:], db1_d[:])
            W1_s = wp.tile([F_IN, F_HID], f32)
            nc.sync.dma_start(W1_s[:], W1_d[:])
            W2_s = wp.tile([F_HID, F_OUT], f32)
            nc.sync.dma_start(W2_s[:], W2_d[:])
            b2r_s = wp.tile([P, F_OUT], f32)
            nc.sync.dma_start(b2r_s[:], b2r_d[:])
            ident = wp.tile([P, P], f32)
            make_identity(nc, ident[:])
            zt = wp.tile([P, F_HID], f32)
            nc.vector.memset(zt[:], 0.0)
            nc.sync.dma_start(slab1[NsP:NsP + P, :], zt[:])
            nc.sync.dma_start(slab2[NsP:NsP + P, :], zt[:])

            # ---- Phase A: gs1 = dinv * (x @ W1), write slab1 ----
            for g in range(G):
                g1p = pp.tile([P, F_HID], f32, tag="mm1")
                nc.tensor.matmul(g1p[:], lhsT=xT_s[:, g * P:(g + 1) * P],
                                 rhs=W1_s[:], start=True, stop=True)
                gs1 = sb.tile([P, F_HID], f32, tag="gs1")
                nc.vector.tensor_scalar_mul(gs1[:], g1p[:], dinv_s[:, g:g + 1])
                nc.sync.dma_start(slab1[g * P:(g + 1) * P, :], gs1[:])
                if dbg:
                    nc.sync.dma_start(dslab_d[g * P:(g + 1) * P, :], gs1[:])

            nc.gpsimd.collective_compute(
                "AllGather", mybir.AluOpType.bypass, replica_groups=RG,
                ins=[slab1[:]], outs=[tab1[:]])

            if dbg:
                for j in range(16):
                    half = 0 if j < 8 else 1
                    src0 = (j % 8) * P if half == 0 else S + (j % 8) * P
                    dt_t = sb.tile([P, F_HID], f32, tag="dtab")
                    nc.sync.dma_start(dt_t[:], tab1[src0:src0 + P, :])
                    nc.sync.dma_start(dtab_d[j * P:(j + 1) * P, :], dt_t[:])

            # ---- Phase B: s1 = gather-sum + self; gs2 = relu(dinv2*s1 + dinv*b1) ----
            for g in range(G):
                K = Ks[g]
                o = offs[g]
                gt = gp.tile([P, K, F_HID], f32, tag="gath")
                for k in range(K):
                    nc.gpsimd.indirect_dma_start(
                        out=gt[:, k, :], out_offset=None, in_=tab1[:],
                        in_offset=bass.IndirectOffsetOnAxis(
                            ap=idx_s[:, o + k:o + k + 1], axis=0))
                s1 = sb.tile([P, F_HID], f32, tag="s1")
                nc.vector.reduce_sum(out=s1[:], in_=gt[:].rearrange("p k f -> p f k"),
                                     axis=mybir.AxisListType.X)
                sf = sb.tile([P, F_HID], f32, tag="sf")
                nc.sync.dma_start(sf[:], slab1[g * P:(g + 1) * P, :])
                nc.vector.tensor_add(s1[:], s1[:], sf[:])
                if dbg:
                    nc.sync.dma_start(dagg_d[g * P:(g + 1) * P, :], s1[:])
                nc.vector.tensor_scalar_mul(s1[:], s1[:], dinv2_s[:, g:g + 1])
                nc.vector.tensor_add(s1[:], s1[:], db1_s[:, g * F_HID:(g + 1) * F_HID])
                gs2 = sb.tile([P, F_HID], f32, tag="gs2")
                nc.vector.tensor_scalar_max(gs2[:], s1[:], 0.0)
                nc.sync.dma_start(slab2[g * P:(g + 1) * P, :], gs2[:])

            nc.gpsimd.collective_compute(
                "AllGather", mybir.AluOpType.bypass, replica_groups=RG,
                ins=[slab2[:]], outs=[tab2[:]])

            # ---- Phase C: s2 = gather-sum + self; out = (dinv*s2) @ W2 + b2 ----
            for g in range(G):
                K = Ks[g]
                o = offs[g]
                gt = gp.tile([P, K, F_HID], f32, tag="gath")
                for k in range(K):
                    nc.gpsimd.indirect_dma_start(
                        out=gt[:, k, :], out_offset=None, in_=tab2[:],
                        in_offset=bass.IndirectOffsetOnAxis(
                            ap=idx_s[:, o + k:o + k + 1], axis=0))
                s2 = sb.tile([P, F_HID], f32, tag="s2")
                nc.vector.reduce_sum(out=s2[:], in_=gt[:].rearrange("p k f -> p f k"),
                                     axis=mybir.AxisListType.X)
                sf = sb.tile([P, F_HID], f32, tag="sf")
                nc.sync.dma_start(sf[:], slab2[g * P:(g + 1) * P, :])
                nc.vector.tensor_add(s2[:], s2[:], sf[:])
                nc.vector.tensor_scalar_mul(s2[:], s2[:], dinv_s[:, g:g + 1])
                tpp = pp.tile([F_HID, P], f32, tag="tr")
                nc.tensor.transpose(tpp[:], s2[:], ident[:])
                s2T = sb.tile([F_HID, P], f32, tag="s2T")
                nc.vector.tensor_copy(s2T[:], tpp[:])
                op = pp.tile([P, F_OUT], f32, tag="mm2")
                nc.tensor.matmul(op[:], lhsT=s2T[:], rhs=W2_s[:],
                                 start=True, stop=True)
                of = sb.tile([P, F_OUT], f32, tag="of")
                nc.vector.tensor_add(of[:], op[:], b2r_s[:])
                nc.sync.dma_start(out_d[g * P:(g + 1) * P, :], of[:])

    nc.compile()
    return nc


def _assemble(results, meta):
    M = N_CORES
    Ns, N, F_OUT = meta["Ns"], meta["N"], meta["F_OUT"]
    out = np.empty((N, F_OUT), dtype=np.float32)
    for m in range(M):
        pos_of = meta["pos_of_list"][m]
        out[m * Ns:(m + 1) * Ns] = results[m]["out"][pos_of[:Ns]]
    return out


_CACHE = {}


def kernel(x, edge_index, W1, b1, W2, b2):
    meta, in_maps = _preprocess(x, edge_index, W1, b1, W2, b2)
    key = (meta["N"], meta["SUMK"], tuple(meta["Ks"]))
    if key not in _CACHE:
        _CACHE[key] = _build_program(meta)
    nc = _CACHE[key]
    from concourse import bass_utils
    res = bass_utils.run_bass_kernel_spmd(nc, in_maps, core_ids=list(range(N_CORES)))
    return _assemble(res.results, meta)


# revision 11
# speedup vs baseline: 1.0001x; 1.0001x over previous
"""Trainium2 Bass kernel for 2-layer GCN (nn_BasicGNN).

Strategy (8 NeuronCores, SPMD):
  - Reorder layer 2 as (A_norm @ z1) @ W2 so both aggregations move 16 feats.
  - out[v] = dinv[v] * (sum_{u->v} dinv[u]*h[u] + dinv[v]*h[v])  (self-loop)
    => per-node pre-scale by dinv, aggregate raw sums, post-scale by dinv.
  - Shard destinations across 8 cores (12500 each). Host sorts each core's
    dests by in-degree, pads edge lists per 128-dest group to the group max
    K_g, and maps edge sources to "table rows" (per-core slabs concatenated
    by AllGather).
  - Device per layer: gather [128, K_g*16] rows per group with one
    indirect DMA, strided reduce_sum on DVE, add self term, scale, next.
  - Weights (128x16, 16x40) replicated; AllGather shares the 16-feature
    tables between layers.
"""

import sys
import numpy as np

if "/opt/trn_rl_repo" not in sys.path:
    sys.path.insert(0, "/opt/trn_rl_repo")

N_CORES = 8
P = 128


def _preprocess(x, edge_index, W1, b1, W2, b2):
    x = np.asarray(x, dtype=np.float32)
    W1 = np.asarray(W1, dtype=np.float32)
    b1 = np.asarray(b1, dtype=np.float32)
    W2 = np.asarray(W2, dtype=np.float32)
    b2 = np.asarray(b2, dtype=np.float32)
    N, F_IN = x.shape
    F_HID = W1.shape[1]
    F_OUT = W2.shape[1]
    M = N_CORES
    assert N % M == 0
    Ns = N // M
    NsP = ((Ns + P - 1) // P) * P      # padded dest count per core
    G = NsP // P                        # dest groups per core
    S = NsP + P                         # slab rows (last P rows = zeros)
    PAD_ROW = NsP                       # table row that is guaranteed zero (core 0 zero block)

    row = np.asarray(edge_index[0]).astype(np.int64)
    col = np.asarray(edge_index[1]).astype(np.int64)
    deg = np.bincount(col, minlength=N).astype(np.int64) + 1
    dinv = (deg.astype(np.float64) ** -0.5).astype(np.float32)

    # Per-core degree-sorted dest permutation; node -> global table row
    pos_global = np.empty(N, dtype=np.int64)
    pos_of_list = []
    sorted_indeg = []
    for m in range(M):
        indeg = deg[m * Ns:(m + 1) * Ns] - 1
        key = np.concatenate([indeg, np.full(NsP - Ns, -1, dtype=np.int64)])
        order = np.argsort(key, kind="stable")
        pos_of = np.empty(NsP, dtype=np.int64)
        pos_of[order] = np.arange(NsP)
        pos_of_list.append(pos_of)
        sorted_indeg.append(np.maximum(key[order], 0))
        pos_global[m * Ns:(m + 1) * Ns] = m * S + pos_of[:Ns]

    # Shared per-group K (max over cores, >=1)
    Ks = np.zeros(G, dtype=np.int64)
    for m in range(M):
        si = sorted_indeg[m].reshape(G, P)
        Ks = np.maximum(Ks, si.max(axis=1))
    Ks = np.maximum(Ks, 1)
    offs = np.zeros(G + 1, dtype=np.int64)
    offs[1:] = np.cumsum(Ks)
    SUMK = int(offs[-1])

    in_maps = []
    for m in range(M):
        pos_of = pos_of_list[m]
        mask = (col >= m * Ns) & (col < (m + 1) * Ns)
        er = row[mask]
        dpos = pos_of[col[mask] - m * Ns]
        src_row = pos_global[er]
        o = np.argsort(dpos, kind="stable")
        dpos = dpos[o]
        src_row = src_row[o]
        cnt = np.bincount(dpos, minlength=NsP)
        starts = np.concatenate([[0], np.cumsum(cnt)])[:-1]
        rank = np.arange(len(dpos)) - starts[dpos]
        idx_all = np.full((P, SUMK), PAD_ROW, dtype=np.int32)
        g_of = dpos // P
        p_of = dpos % P
        idx_all[p_of, offs[g_of] + rank] = src_row.astype(np.int32)

        xp = np.zeros((NsP, F_IN), np.float32)
        xp[pos_of[:Ns]] = x[m * Ns:(m + 1) * Ns]
        xT = np.ascontiguousarray(xp.T)

        d_sorted = np.ones(NsP, np.float32)
        d_sorted[pos_of[:Ns]] = dinv[m * Ns:(m + 1) * Ns]
        dinv_col = np.ascontiguousarray(d_sorted.reshape(G, P).T)
        dinv2_col = np.ascontiguousarray(dinv_col * dinv_col)
        db1 = (d_sorted.reshape(G, P)[:, :, None] * b1[None, None, :])
        db1 = np.ascontiguousarray(db1.transpose(1, 0, 2).reshape(P, G * F_HID))

        in_maps.append({
            "xT": xT,
            "idx": idx_all,
            "dinv": dinv_col,
            "dinv2": dinv2_col,
            "db1": db1.astype(np.float32),
            "W1": W1,
            "W2": W2,
            "b2r": np.ascontiguousarray(np.tile(b2[None, :], (P, 1))),
        })

    meta = dict(N=N, Ns=Ns, NsP=NsP, G=G, S=S, Ks=Ks.tolist(), offs=offs.tolist(),
                SUMK=SUMK, F_IN=F_IN, F_HID=F_HID, F_OUT=F_OUT,
                pos_of_list=pos_of_list, b1_zero=bool(not np.any(b1)))
    return meta, in_maps


def _build_program(meta, dbg=False):
    import concourse.bacc as bacc
    import concourse.tile as tile
    import concourse.bass as bass
    import concourse.mybir as mybir
    from concourse.masks import make_identity

    f32 = mybir.dt.float32
    i32 = mybir.dt.int32
    G, S, NsP, SUMK = meta["G"], meta["S"], meta["NsP"], meta["SUMK"]
    Ks, offs = meta["Ks"], meta["offs"]
    F_IN, F_HID, F_OUT = meta["F_IN"], meta["F_HID"], meta["F_OUT"]
    M = N_CORES

    nc = bacc.Bacc("TRN2", target_bir_lowering=False, debug=False,
                   enable_asserts=False, num_devices=M)

    xT_d = nc.dram_tensor("xT", [P, NsP], f32, kind="ExternalInput")
    idx_d = nc.dram_tensor("idx", [P, SUMK], i32, kind="ExternalInput")
    dinv_d = nc.dram_tensor("dinv", [P, G], f32, kind="ExternalInput")
    dinv2_d = nc.dram_tensor("dinv2", [P, G], f32, kind="ExternalInput")
    db1_d = nc.dram_tensor("db1", [P, G * F_HID], f32, kind="ExternalInput")
    W1_d = nc.dram_tensor("W1", [F_IN, F_HID], f32, kind="ExternalInput")
    W2_d = nc.dram_tensor("W2", [F_HID, F_OUT], f32, kind="ExternalInput")
    b2r_d = nc.dram_tensor("b2r", [P, F_OUT], f32, kind="ExternalInput")
    out_d = nc.dram_tensor("out", [NsP, F_OUT], f32, kind="ExternalOutput")

    if dbg:
        dslab_d = nc.dram_tensor("dslab", [NsP, F_HID], f32, kind="ExternalOutput")
        dtab_d = nc.dram_tensor("dtab", [2048, F_HID], f32, kind="ExternalOutput")
        dagg_d = nc.dram_tensor("dagg", [NsP, F_HID], f32, kind="ExternalOutput")
    slab1 = nc.dram_tensor("slab1", [S, F_HID], f32, kind="Internal")
    slab2 = nc.dram_tensor("slab2", [S, F_HID], f32, kind="Internal")
    tab1 = nc.dram_tensor("tab1", [M * S, F_HID], f32, kind="Internal",
                          addr_space="Shared")
    tab2 = nc.dram_tensor("tab2", [M * S, F_HID], f32, kind="Internal",
                          addr_space="Shared")
    RG = [list(range(M))]

    with tile.TileContext(nc) as tc:
        with tc.tile_pool(name="big", bufs=1) as bigp, \
             tc.tile_pool(name="wts", bufs=1) as wp, \
             tc.tile_pool(name="work", bufs=8) as sb, \
             tc.tile_pool(name="gath", bufs=6) as gp, \
             tc.tile_pool(name="ps", bufs=2, space="PSUM") as pp:

            xT_s = bigp.tile([P, NsP], f32)
            nc.sync.dma_start(xT_s[:], xT_d[:])
            idx_s = bigp.tile([P, SUMK], i32)
            nc.sync.dma_start(idx_s[:], idx_d[:])
            dinv_s = wp.tile([P, G], f32)
            nc.sync.dma_start(dinv_s[:], dinv_d[:])
            dinv2_s = wp.tile([P, G], f32)
            nc.sync.dma_start(dinv2_s[:], dinv2_d[:])
            db1_s = wp.tile([P, G * F_HID], f32)
            nc.sync.dma_start(db1_s[:], db1_d[:])
            W1_s = wp.tile([F_IN, F_HID], f32)
            nc.sync.dma_start(W1_s[:], W1_d[:])
            W2_s = wp.tile([F_HID, F_OUT], f32)
            nc.sync.dma_start(W2_s[:], W2_d[:])
            b2r_s = wp.tile([P, F_OUT], f32)
            nc.sync.dma_start(b2r_s[:], b2r_d[:])
            ident = wp.tile([P, P], f32)
            make_identity(nc, ident[:])
            zt = wp.tile([P, F_HID], f32)
            nc.vector.memset(zt[:], 0.0)
            nc.sync.dma_start(slab1[NsP:NsP + P, :], zt[:])
            nc.sync.dma_start(slab2[NsP:NsP + P, :], zt[:])

            # ---- Phase A: gs1 = dinv * (x @ W1), write slab1 ----
            for g in range(G):
                g1p = pp.tile([P, F_HID], f32, tag="mm1")
                nc.tensor.matmul(g1p[:], lhsT=xT_s[:, g * P:(g + 1) * P],
                                 rhs=W1_s[:], start=True, stop=True)
                gs1 = sb.tile([P, F_HID], f32, tag="gs1")
                nc.vector.tensor_scalar_mul(gs1[:], g1p[:], dinv_s[:, g:g + 1])
                nc.sync.dma_start(slab1[g * P:(g + 1) * P, :], gs1[:])
                if dbg:
                    nc.sync.dma_start(dslab_d[g * P:(g + 1) * P, :], gs1[:])

            nc.gpsimd.collective_compute(
                "AllGather", mybir.AluOpType.bypass, replica_groups=RG,
                ins=[slab1[:]], outs=[tab1[:]])

            if dbg:
                for j in range(16):
                    half = 0 if j < 8 else 1
                    src0 = (j % 8) * P if half == 0 else S + (j % 8) * P
                    dt_t = sb.tile([P, F_HID], f32, tag="dtab")
                    nc.sync.dma_start(dt_t[:], tab1[src0:src0 + P, :])
                    nc.sync.dma_start(dtab_d[j * P:(j + 1) * P, :], dt_t[:])

            # ---- Phase B: s1 = gather-sum + self; gs2 = relu(dinv2*s1 + dinv*b1) ----
            for g in range(G):
                K = Ks[g]
                o = offs[g]
                gt = gp.tile([P, K, F_HID], f32, tag="gath")
                for k in range(K):
                    nc.gpsimd.indirect_dma_start(
                        out=gt[:, k, :], out_offset=None, in_=tab1[:],
                        in_offset=bass.IndirectOffsetOnAxis(
                            ap=idx_s[:, o + k:o + k + 1], axis=0))
                s1 = sb.tile([P, F_HID], f32, tag="s1")
                nc.vector.reduce_sum(out=s1[:], in_=gt[:].rearrange("p k f -> p f k"),
                                     axis=mybir.AxisListType.X)
                sf = sb.tile([P, F_HID], f32, tag="sf")
                nc.sync.dma_start(sf[:], slab1[g * P:(g + 1) * P, :])
                nc.vector.tensor_add(s1[:], s1[:], sf[:])
                if dbg:
                    nc.sync.dma_start(dagg_d[g * P:(g + 1) * P, :], s1[:])
                gs2 = sb.tile([P, F_HID], f32, tag="gs2")
                if meta.get("b1_zero"):
                    nc.vector.tensor_scalar(
                        out=gs2[:], in0=s1[:], scalar1=dinv2_s[:, g:g + 1],
                        scalar2=0.0, op0=mybir.AluOpType.mult,
                        op1=mybir.AluOpType.max)
                else:
                    nc.vector.tensor_scalar_mul(s1[:], s1[:], dinv2_s[:, g:g + 1])
                    nc.vector.tensor_add(s1[:], s1[:],
                                         db1_s[:, g * F_HID:(g + 1) * F_HID])
                    nc.vector.tensor_scalar_max(gs2[:], s1[:], 0.0)
                nc.sync.dma_start(slab2[g * P:(g + 1) * P, :], gs2[:])

            nc.gpsimd.collective_compute(
                "AllGather", mybir.AluOpType.bypass, replica_groups=RG,
                ins=[slab2[:]], outs=[tab2[:]])

            # ---- Phase C: s2 = gather-sum + self; out = (dinv*s2) @ W2 + b2 ----
            for g in range(G):
                K = Ks[g]
                o = offs[g]
                gt = gp.tile([P, K, F_HID], f32, tag="gath")
                for k in range(K):
                    nc.gpsimd.indirect_dma_start(
                        out=gt[:, k, :], out_offset=None, in_=tab2[:],
                        in_offset=bass.IndirectOffsetOnAxis(
                            ap=idx_s[:, o + k:o + k + 1], axis=0))
                s2 = sb.tile([P, F_HID], f32, tag="s2")
                nc.vector.reduce_sum(out=s2[:], in_=gt[:].rearrange("p k f -> p f k"),
                                     axis=mybir.AxisListType.X)
                sf = sb.tile([P, F_HID], f32, tag="sf")
                nc.sync.dma_start(sf[:], slab2[g * P:(g + 1) * P, :])
                nc.vector.tensor_add(s2[:], s2[:], sf[:])
                nc.vector.tensor_scalar_mul(s2[:], s2[:], dinv_s[:, g:g + 1])
                tpp = pp.tile([F_HID, P], f32, tag="tr")
                nc.tensor.transpose(tpp[:], s2[:], ident[:])
                s2T = sb.tile([F_HID, P], f32, tag="s2T")
                nc.vector.tensor_copy(s2T[:], tpp[:])
                op = pp.tile([P, F_OUT], f32, tag="mm2")
                nc.tensor.matmul(op[:], lhsT=s2T[:], rhs=W2_s[:],
                                 start=True, stop=True)
                of = sb.tile([P, F_OUT], f32, tag="of")
                nc.vector.tensor_add(of[:], op[:], b2r_s[:])
                nc.sync.dma_start(out_d[g * P:(g + 1) * P, :], of[:])

    nc.compile()
    return nc


def _assemble(results, meta):
    M = N_CORES
    Ns, N, F_OUT = meta["Ns"], meta["N"], meta["F_OUT"]
    out = np.empty((N, F_OUT), dtype=np.float32)
    for m in range(M):
        pos_of = meta["pos_of_list"][m]
        out[m * Ns:(m + 1) * Ns] = results[m]["out"][pos_of[:Ns]]
    return out


_CACHE = {}


def kernel(x, edge_index, W1, b1, W2, b2):
    meta, in_maps = _preprocess(x, edge_index, W1, b1, W2, b2)
    key = (meta["N"], meta["SUMK"], tuple(meta["Ks"]))
    if key not in _CACHE:
        _CACHE[key] = _build_program(meta)
    nc = _CACHE[key]
    from concourse import bass_utils
    res = bass_utils.run_bass_kernel_spmd(nc, in_maps, core_ids=list(range(N_CORES)))
    return _assemble(res.results, meta)
